# revision 23
# baseline (speedup 1.0000x reference)
"""Masked-softmax complementary-label loss on 8 Trainium2 NeuronCores.

Strategy (pure data parallel, hardcoded for B=32768, C=1000, K=10):
  - Shard batch across 8 cores (4096 rows each).
  - Each core streams its [4096, 1000] f32 logit shard through SBUF and
    computes per-row Z = sum_c exp(x[r, c]): exp on the scalar engine
    (in place), row sums on the vector engine, with the final tile using
    the ACT accumulator to minimize the post-stream tail (memory-bound).
  - Host gathers the 10 complementary-label logits per row (tiny),
    dedups duplicate labels, and finishes the per-row loss:
        S   = sum_k w_k * exp(g_k)          (w: first-occurrence weights)
        p_nc = (Z - S) / Z                  (probability mass not in set)
        loss = -log(p_nc + 1e-7)
        out  = mean(scale * loss),  scale = (C-1)/(C - num_comp)
"""

import numpy as np

B = 32768
C = 1000
K = 10
NCORES = 8
B_LOCAL = B // NCORES  # 4096
P = 128
NT = B_LOCAL // P  # 32 row-tiles of 128 rows per core
EPS = 1e-7

_PROG_CACHE = {}


def _build_program():
    """Build the single-core Bass program (SPMD across 8 cores).

    Raw Bass (no TileContext): this toolchain's walrus rejects instructions
    with more than a couple of embedded sync-wait commands, which Tile's
    scheduler and tail drain freely emit. With manual semaphores every wait
    is its own sequencer instruction, so there is no such limit.

    Layout: the whole 16 MB shard stays resident in SBUF (125 KB of the
    192 KB partition budget), so load DMAs have no WAR hazards at all.
    """
    from contextlib import ExitStack

    import concourse.bass as bass
    from concourse import mybir

    nc = bass.Bass(
        "TRN2", target_bir_lowering=False, debug=False, num_devices=NCORES
    )
    x = nc.dram_tensor(
        "x", [B_LOCAL, C], mybir.dt.float32, kind="ExternalInput"
    ).ap()
    z = nc.dram_tensor(
        "z", [P, NT + 1], mybir.dt.float32, kind="ExternalOutput"
    ).ap()
    x3 = x.rearrange("(n p) c -> n p c", p=P)  # [NT, P, C]

    # DMA chunk schedule, in row-tiles (sum = NT).  1 MB transfers for the
    # bulk (full 360 GB/s), then a progressively smaller tail so the final
    # exp/sum work after the last byte is minimal.  The last tile is split
    # into two half-tile (500-col) chunks handled by the ACT accumulator.
    chunks = [2] * ((NT - 2) // 2) + [1]  # tiles 0..NT-2
    assert sum(chunks) == NT - 1

    nchunks = len(chunks) + 2  # data chunks + two half-tile chunks

    with (
        nc.sbuf_tensor([P, NT * C], mybir.dt.float32) as xbuf,
        nc.sbuf_tensor([P, NT + 1], mybir.dt.float32) as ztile,
        nc.sbuf_tensor([1, 1], mybir.dt.float32) as fence,
        ExitStack() as stack,
        nc.semaphore() as act_sem,
        nc.semaphore() as red_sem,
        nc.semaphore() as out_sem,
        nc.Block() as block,
    ):
        # One semaphore per load chunk.  A single shared counter is UNSAFE
        # with >1 DMA in flight: each of the 16 SDMA engines increments +1
        # after finishing its own slice, in FIFO order per engine — so fast
        # engines can run ahead into later chunks and push the shared count
        # past 16*(j+1) while a slow engine still owes chunk j its slice.
        in_sems = [
            stack.enter_context(nc.semaphore(f"in{j}")) for j in range(nchunks)
        ]
        H = C // 2  # half-tile columns

        @block.sync
        def _(sp):
            t = 0
            for j, w in enumerate(chunks):
                src = x3[t : t + w].rearrange("n p c -> p n c")
                dst = xbuf[:, t * C : (t + w) * C].rearrange(
                    "p (n c) -> p n c", c=C
                )
                sp.dma_start(dst, src).then_inc(in_sems[j], 16)
                t += w
            # last tile in two half-tile chunks
            base = (NT - 1) * C
            for h in range(2):
                sp.dma_start(
                    xbuf[:, base + h * H : base + (h + 1) * H],
                    x3[NT - 1, :, h * H : (h + 1) * H],
                ).then_inc(in_sems[len(chunks) + h], 16)
            # DVE reduced tiles 0..NT-2 (one op per chunk); ACT accum covered
            # the last tile as two partial sums (columns NT-1 and NT).  The
            # +3rd inc comes from a trailing ACT fence op: the accum
            # ACTIVATE's own then_inc fires before walrus's READ_ACCUMULATOR
            # writes ztile, so waiting on it directly races the store.
            sp.wait_ge(red_sem, len(chunks))
            sp.wait_ge(act_sem, len(chunks) + 3)
            sp.dma_start(z, ztile[:]).then_inc(out_sem, 16)
            sp.wait_ge(out_sem, 16)

        @block.scalar
        def _(act):
            # One batched in-place exp per DMA chunk (amortizes the 352-cycle
            # ACTIVATE pipeline cost); row-sums on DVE except the final tile,
            # where the ACT accumulator (0.28us) beats a DVE reduce (1.2us).
            done = 0
            for j, w in enumerate(chunks):
                act.wait_ge(in_sems[j], 16)
                sub = xbuf[:, done * C : (done + w) * C]
                act.activation(
                    sub, sub, mybir.ActivationFunctionType.Exp
                ).then_inc(act_sem, 1)
                done += w
            base = (NT - 1) * C
            for h in range(2):
                act.wait_ge(in_sems[len(chunks) + h], 16)
                sub = xbuf[:, base + h * H : base + (h + 1) * H]
                act.activation(
                    sub,
                    sub,
                    mybir.ActivationFunctionType.Exp,
                    accum_out=ztile[:, NT - 1 + h : NT + h],
                ).then_inc(act_sem, 1)
            # same-engine ordering: this tiny engine op completes only after
            # the READ_ACCUMULATOR stores above have retired
            act.mul(fence[:], fence[:], 0.0).then_inc(act_sem, 1)

        @block.vector
        def _(dve):
            # one batched reduce per exp'd chunk
            done = 0
            for k, w in enumerate(chunks):
                dve.wait_ge(act_sem, k + 1)
                dve.tensor_reduce(
                    ztile[:, done : done + w],
                    xbuf[:, done * C : (done + w) * C].rearrange(
                        "p (n c) -> p n c", c=C
                    ),
                    axis=mybir.AxisListType.X,
                    op=mybir.AluOpType.add,
                ).then_inc(red_sem, 1)
                done += w

    return nc


def _get_program():
    if "nc" not in _PROG_CACHE:
        _PROG_CACHE["nc"] = _build_program()
    return _PROG_CACHE["nc"]


def run_device(outputs_np, trace=False, trace_kwargs=None):
    """Run the Bass kernel on 8 cores; returns (Z[B] float32, BassKernelResults)."""
    from concourse.bass_utils import run_bass_kernel_spmd

    nc = _get_program()
    in_maps = [
        {"x": np.ascontiguousarray(outputs_np[r * B_LOCAL : (r + 1) * B_LOCAL])}
        for r in range(NCORES)
    ]
    kw = {}
    if trace:
        kw["trace"] = True
        if trace_kwargs:
            kw["trace_kwargs"] = trace_kwargs
    res = run_bass_kernel_spmd(nc, in_maps, list(range(NCORES)), **kw)
    zs = []
    for r in range(NCORES):
        zr = np.asarray(res.results[r]["z"])  # [P, NT+1]
        # last tile's sum arrives as two half-tile partial sums
        zfull = zr[:, :NT].copy()
        zfull[:, NT - 1] += zr[:, NT]
        zs.append(zfull)
    # z[p, i] corresponds to shard row i*P + p
    Z = np.concatenate([z.T.reshape(-1) for z in zs])  # [B]
    return Z, res


def _host_label_prep(outputs_np, labels_np):
    """Dedup weights, gathered logits, and per-row scale from the labels."""
    labels = labels_np.astype(np.int64)
    valid = labels != -1  # [B, K]
    num_comp = valid.sum(axis=1)  # [B]
    # first-occurrence mask: entry k is a dup if some j < k holds same value
    eq = labels[:, :, None] == labels[:, None, :]  # [B, K, K]
    earlier = np.arange(K)[None, :] < np.arange(K)[:, None]  # [K, K], (k, j): j<k
    is_dup = (eq & earlier[None, :, :]).any(axis=2)  # [B, K]
    w = valid & ~is_dup  # [B, K] bool
    safe = np.where(valid, labels, 0)
    g = outputs_np[np.arange(B)[:, None], safe]  # [B, K] f32 gathered logits
    return w, g, num_comp


def finish_loss(Z, w, g, num_comp):
    S = np.where(w, np.exp(g.astype(np.float64)), 0.0).sum(axis=1)  # [B]
    Z64 = Z.astype(np.float64)
    p_nc = (Z64 - S) / Z64
    loss = -np.log(p_nc + EPS)
    scale = (C - 1) / (C - num_comp.astype(np.float64))
    return np.asarray((scale * loss).mean(), dtype=np.float32)


def kernel(**inputs):
    outputs_np = np.ascontiguousarray(
        np.asarray(inputs["outputs"], dtype=np.float32)
    )
    labels_np = np.asarray(inputs["complementary_labels"])
    assert outputs_np.shape == (B, C)
    assert labels_np.shape == (B, K)

    w, g, num_comp = _host_label_prep(outputs_np, labels_np)
    Z, _ = run_device(outputs_np)
    return finish_loss(Z, w, g, num_comp)


# revision 26
# speedup vs baseline: 1.1607x; 1.1607x over previous
"""Masked-softmax complementary-label loss on 8 Trainium2 NeuronCores.

Strategy (pure data parallel, hardcoded for B=32768, C=1000, K=10):
  - Shard batch across 8 cores (4096 rows each).
  - Each core streams its [4096, 1000] f32 logit shard through SBUF and
    computes per-row Z = sum_c exp(x[r, c]): exp on the scalar engine
    (in place), row sums on the vector engine, with the final tile using
    the ACT accumulator to minimize the post-stream tail (memory-bound).
  - Host gathers the 10 complementary-label logits per row (tiny),
    dedups duplicate labels, and finishes the per-row loss:
        S   = sum_k w_k * exp(g_k)          (w: first-occurrence weights)
        p_nc = (Z - S) / Z                  (probability mass not in set)
        loss = -log(p_nc + 1e-7)
        out  = mean(scale * loss),  scale = (C-1)/(C - num_comp)
"""

import numpy as np

B = 32768
C = 1000
K = 10
NCORES = 8
B_LOCAL = B // NCORES  # 4096
P = 128
NT = B_LOCAL // P  # 32 row-tiles of 128 rows per core
EPS = 1e-7

_PROG_CACHE = {}


def _build_program():
    """Build the single-core Bass program (SPMD across 8 cores).

    Raw Bass (no TileContext): this toolchain's walrus rejects instructions
    with more than a couple of embedded sync-wait commands, which Tile's
    scheduler and tail drain freely emit. With manual semaphores every wait
    is its own sequencer instruction, so there is no such limit.

    Layout: the whole 16 MB shard stays resident in SBUF (125 KB of the
    192 KB partition budget), so load DMAs have no WAR hazards at all.
    """
    from contextlib import ExitStack

    import concourse.bass as bass
    from concourse import mybir

    nc = bass.Bass(
        "TRN2", target_bir_lowering=False, debug=False, num_devices=NCORES
    )

    # Strip the const-AP pool init (4 gpsimd memsets) and the all-engine
    # barrier bass unconditionally emits before user code: the barrier sits
    # on the critical path to the first DMA (~0.6us), and we never read the
    # const pool (the Exp bias below is our own AP, zeroed on ACT itself).
    b0 = nc.m.functions[0].blocks[0]
    insts = list(b0.instructions)
    first_const = next(
        i
        for i, ins in enumerate(insts)
        if type(ins).__name__ == "InstMemset"
        and "const-" in str(ins.outs[0].concise())
    )
    dropped = insts[first_const:]
    assert all(
        type(d).__name__ in ("InstMemset", "InstDrain", "InstEventSemaphore")
        for d in dropped
    ), [type(d).__name__ for d in dropped]
    b0.instructions = insts[:first_const]

    x = nc.dram_tensor(
        "x", [B_LOCAL, C], mybir.dt.float32, kind="ExternalInput"
    ).ap()
    z = nc.dram_tensor(
        "z", [P, NT + 1], mybir.dt.float32, kind="ExternalOutput"
    ).ap()
    x3 = x.rearrange("(n p) c -> n p c", p=P)  # [NT, P, C]

    # DMA chunk schedule, in row-tiles (sum = NT).  1 MB transfers for the
    # bulk (full 360 GB/s), then a progressively smaller tail so the final
    # exp/sum work after the last byte is minimal.  The last tile is split
    # into two half-tile (500-col) chunks handled by the ACT accumulator.
    chunks = [2] * ((NT - 2) // 2) + [1]  # tiles 0..NT-2
    assert sum(chunks) == NT - 1

    nchunks = len(chunks) + 2  # data chunks + two half-tile chunks

    with (
        nc.sbuf_tensor([P, NT * C], mybir.dt.float32) as xbuf,
        nc.sbuf_tensor([P, NT + 1], mybir.dt.float32) as ztile,
        nc.sbuf_tensor([1, 1], mybir.dt.float32) as fence,
        nc.sbuf_tensor([P, 1], mybir.dt.float32) as bias0,
        ExitStack() as stack,
        nc.semaphore() as act_sem,
        nc.semaphore() as red_sem,
        nc.semaphore() as out_sem,
        nc.Block() as block,
    ):
        # One semaphore per load chunk.  A single shared counter is UNSAFE
        # with >1 DMA in flight: each of the 16 SDMA engines increments +1
        # after finishing its own slice, in FIFO order per engine — so fast
        # engines can run ahead into later chunks and push the shared count
        # past 16*(j+1) while a slow engine still owes chunk j its slice.
        in_sems = [
            stack.enter_context(nc.semaphore(f"in{j}")) for j in range(nchunks)
        ]
        H = C // 2  # half-tile columns

        @block.sync
        def _(sp):
            t = 0
            for j, w in enumerate(chunks):
                src = x3[t : t + w].rearrange("n p c -> p n c")
                dst = xbuf[:, t * C : (t + w) * C].rearrange(
                    "p (n c) -> p n c", c=C
                )
                sp.dma_start(dst, src).then_inc(in_sems[j], 16)
                t += w
            # last tile in two half-tile chunks
            base = (NT - 1) * C
            for h in range(2):
                sp.dma_start(
                    xbuf[:, base + h * H : base + (h + 1) * H],
                    x3[NT - 1, :, h * H : (h + 1) * H],
                ).then_inc(in_sems[len(chunks) + h], 16)
            # DVE reduced tiles 0..NT-2 (one op per chunk); ACT accum covered
            # the last tile as two partial sums (columns NT-1 and NT).  The
            # +3rd inc comes from a trailing ACT fence op: the accum
            # ACTIVATE's own then_inc fires before walrus's READ_ACCUMULATOR
            # writes ztile, so waiting on it directly races the store.
            sp.wait_ge(red_sem, len(chunks))
            sp.wait_ge(act_sem, len(chunks) + 3)
            sp.dma_start(z, ztile[:]).then_inc(out_sem, 16)
            sp.wait_ge(out_sem, 16)

        @block.scalar
        def _(act):
            # One batched in-place exp per DMA chunk (amortizes the 352-cycle
            # ACTIVATE pipeline cost); row-sums on DVE except the final tile,
            # where the ACT accumulator (0.28us) beats a DVE reduce (1.2us).
            # zero our own bias AP on ACT itself (replaces the const pool)
            act.memzero(bias0[:])
            done = 0
            for j, w in enumerate(chunks):
                act.wait_ge(in_sems[j], 16)
                sub = xbuf[:, done * C : (done + w) * C]
                act.activation(
                    sub, sub, mybir.ActivationFunctionType.Exp, bias=bias0[:]
                ).then_inc(act_sem, 1)
                done += w
            base = (NT - 1) * C
            for h in range(2):
                act.wait_ge(in_sems[len(chunks) + h], 16)
                sub = xbuf[:, base + h * H : base + (h + 1) * H]
                act.activation(
                    sub,
                    sub,
                    mybir.ActivationFunctionType.Exp,
                    bias=bias0[:],
                    accum_out=ztile[:, NT - 1 + h : NT + h],
                ).then_inc(act_sem, 1)
            # same-engine ordering: this tiny engine op completes only after
            # the READ_ACCUMULATOR stores above have retired
            act.mul(fence[:], fence[:], 0.0).then_inc(act_sem, 1)

        @block.vector
        def _(dve):
            # one batched reduce per exp'd chunk
            done = 0
            for k, w in enumerate(chunks):
                dve.wait_ge(act_sem, k + 1)
                dve.tensor_reduce(
                    ztile[:, done : done + w],
                    xbuf[:, done * C : (done + w) * C].rearrange(
                        "p (n c) -> p n c", c=C
                    ),
                    axis=mybir.AxisListType.X,
                    op=mybir.AluOpType.add,
                ).then_inc(red_sem, 1)
                done += w

    return nc


def _get_program():
    if "nc" not in _PROG_CACHE:
        _PROG_CACHE["nc"] = _build_program()
    return _PROG_CACHE["nc"]


def run_device(outputs_np, trace=False, trace_kwargs=None):
    """Run the Bass kernel on 8 cores; returns (Z[B] float32, BassKernelResults)."""
    from concourse.bass_utils import run_bass_kernel_spmd

    nc = _get_program()
    in_maps = [
        {"x": np.ascontiguousarray(outputs_np[r * B_LOCAL : (r + 1) * B_LOCAL])}
        for r in range(NCORES)
    ]
    kw = {}
    if trace:
        kw["trace"] = True
        if trace_kwargs:
            kw["trace_kwargs"] = trace_kwargs
    res = run_bass_kernel_spmd(nc, in_maps, list(range(NCORES)), **kw)
    zs = []
    for r in range(NCORES):
        zr = np.asarray(res.results[r]["z"])  # [P, NT+1]
        # last tile's sum arrives as two half-tile partial sums
        zfull = zr[:, :NT].copy()
        zfull[:, NT - 1] += zr[:, NT]
        zs.append(zfull)
    # z[p, i] corresponds to shard row i*P + p
    Z = np.concatenate([z.T.reshape(-1) for z in zs])  # [B]
    return Z, res


def _host_label_prep(outputs_np, labels_np):
    """Dedup weights, gathered logits, and per-row scale from the labels."""
    labels = labels_np.astype(np.int64)
    valid = labels != -1  # [B, K]
    num_comp = valid.sum(axis=1)  # [B]
    # first-occurrence mask: entry k is a dup if some j < k holds same value
    eq = labels[:, :, None] == labels[:, None, :]  # [B, K, K]
    earlier = np.arange(K)[None, :] < np.arange(K)[:, None]  # [K, K], (k, j): j<k
    is_dup = (eq & earlier[None, :, :]).any(axis=2)  # [B, K]
    w = valid & ~is_dup  # [B, K] bool
    safe = np.where(valid, labels, 0)
    g = outputs_np[np.arange(B)[:, None], safe]  # [B, K] f32 gathered logits
    return w, g, num_comp


def finish_loss(Z, w, g, num_comp):
    S = np.where(w, np.exp(g.astype(np.float64)), 0.0).sum(axis=1)  # [B]
    Z64 = Z.astype(np.float64)
    p_nc = (Z64 - S) / Z64
    loss = -np.log(p_nc + EPS)
    scale = (C - 1) / (C - num_comp.astype(np.float64))
    return np.asarray((scale * loss).mean(), dtype=np.float32)


def kernel(**inputs):
    outputs_np = np.ascontiguousarray(
        np.asarray(inputs["outputs"], dtype=np.float32)
    )
    labels_np = np.asarray(inputs["complementary_labels"])
    assert outputs_np.shape == (B, C)
    assert labels_np.shape == (B, K)

    w, g, num_comp = _host_label_prep(outputs_np, labels_np)
    Z, _ = run_device(outputs_np)
    return finish_loss(Z, w, g, num_comp)


# revision 29
# speedup vs baseline: 1.1702x; 1.0082x over previous
"""Masked-softmax complementary-label loss on 8 Trainium2 NeuronCores.

Strategy (pure data parallel, hardcoded for B=32768, C=1000, K=10):
  - Shard batch across 8 cores (4096 rows each).
  - Each core streams its [4096, 1000] f32 logit shard through SBUF and
    computes per-row Z = sum_c exp(x[r, c]): exp on the scalar engine
    (in place), row sums on the vector engine, with the final tile using
    the ACT accumulator to minimize the post-stream tail (memory-bound).
  - Host gathers the 10 complementary-label logits per row (tiny),
    dedups duplicate labels, and finishes the per-row loss:
        S   = sum_k w_k * exp(g_k)          (w: first-occurrence weights)
        p_nc = (Z - S) / Z                  (probability mass not in set)
        loss = -log(p_nc + 1e-7)
        out  = mean(scale * loss),  scale = (C-1)/(C - num_comp)
"""

import numpy as np

B = 32768
C = 1000
K = 10
NCORES = 8
B_LOCAL = B // NCORES  # 4096
P = 128
NT = B_LOCAL // P  # 32 row-tiles of 128 rows per core
EPS = 1e-7

_PROG_CACHE = {}


def _build_program():
    """Build the single-core Bass program (SPMD across 8 cores).

    Raw Bass (no TileContext): this toolchain's walrus rejects instructions
    with more than a couple of embedded sync-wait commands, which Tile's
    scheduler and tail drain freely emit. With manual semaphores every wait
    is its own sequencer instruction, so there is no such limit.

    Layout: the whole 16 MB shard stays resident in SBUF (125 KB of the
    192 KB partition budget), so load DMAs have no WAR hazards at all.
    """
    from contextlib import ExitStack

    import concourse.bass as bass
    from concourse import mybir

    nc = bass.Bass(
        "TRN2", target_bir_lowering=False, debug=False, num_devices=NCORES
    )

    # Strip the const-AP pool init (4 gpsimd memsets) and the all-engine
    # barrier bass unconditionally emits before user code: the barrier sits
    # on the critical path to the first DMA (~0.6us), and we never read the
    # const pool (the Exp bias below is our own AP, zeroed on ACT itself).
    b0 = nc.m.functions[0].blocks[0]
    insts = list(b0.instructions)
    first_const = next(
        i
        for i, ins in enumerate(insts)
        if type(ins).__name__ == "InstMemset"
        and "const-" in str(ins.outs[0].concise())
    )
    dropped = insts[first_const:]
    assert all(
        type(d).__name__ in ("InstMemset", "InstDrain", "InstEventSemaphore")
        for d in dropped
    ), [type(d).__name__ for d in dropped]
    b0.instructions = insts[:first_const]

    x = nc.dram_tensor(
        "x", [B_LOCAL, C], mybir.dt.float32, kind="ExternalInput"
    ).ap()
    z = nc.dram_tensor(
        "z", [P, NT + 1], mybir.dt.float32, kind="ExternalOutput"
    ).ap()
    x3 = x.rearrange("(n p) c -> n p c", p=P)  # [NT, P, C]

    # DMA chunk schedule, in row-tiles (sum = NT).  1 MB transfers for the
    # bulk (full 360 GB/s), then a progressively smaller tail so the final
    # exp/sum work after the last byte is minimal.  The last tile is split
    # into two half-tile (500-col) chunks handled by the ACT accumulator.
    # Single-tile chunks for the last three data tiles: a trailing 1 MB
    # pair-chunk's exp (1.96us) would otherwise backlog ACT right when the
    # final half-tiles arrive.
    chunks = [2] * ((NT - 4) // 2) + [1, 1, 1]  # tiles 0..NT-2
    assert sum(chunks) == NT - 1

    # last tile arrives as two half-tile pieces (col offset, width);
    # more/smaller pieces lose: each extra accumulator piece serializes
    # another 0.28us READ_ACCUMULATOR between the exps.
    pieces = [(0, 500), (500, 500)]
    nchunks = len(chunks) + len(pieces)

    with (
        nc.sbuf_tensor([P, NT * C], mybir.dt.float32) as xbuf,
        nc.sbuf_tensor([P, NT + 1], mybir.dt.float32) as ztile,
        nc.sbuf_tensor([1, 1], mybir.dt.float32) as fence,
        nc.sbuf_tensor([P, 1], mybir.dt.float32) as bias0,
        ExitStack() as stack,
        nc.semaphore() as act_sem,
        nc.semaphore() as red_sem,
        nc.semaphore() as out_sem,
        nc.Block() as block,
    ):
        # One semaphore per load chunk.  A single shared counter is UNSAFE
        # with >1 DMA in flight: each of the 16 SDMA engines increments +1
        # after finishing its own slice, in FIFO order per engine — so fast
        # engines can run ahead into later chunks and push the shared count
        # past 16*(j+1) while a slow engine still owes chunk j its slice.
        in_sems = [
            stack.enter_context(nc.semaphore(f"in{j}")) for j in range(nchunks)
        ]
        H = C // 2  # half-tile columns

        @block.sync
        def _(sp):
            t = 0
            for j, w in enumerate(chunks):
                src = x3[t : t + w].rearrange("n p c -> p n c")
                dst = xbuf[:, t * C : (t + w) * C].rearrange(
                    "p (n c) -> p n c", c=C
                )
                sp.dma_start(dst, src).then_inc(in_sems[j], 16)
                t += w
            # last tile in tapered pieces
            base = (NT - 1) * C
            for h, (o, wid) in enumerate(pieces):
                sp.dma_start(
                    xbuf[:, base + o : base + o + wid],
                    x3[NT - 1, :, o : o + wid],
                ).then_inc(in_sems[len(chunks) + h], 16)
            # DVE reduced tiles 0..NT-2 (one op per chunk); ACT accum covered
            # the last tile as two partial sums (columns NT-1 and NT).  The
            # +3rd inc comes from a trailing ACT fence op: the accum
            # ACTIVATE's own then_inc fires before walrus's READ_ACCUMULATOR
            # writes ztile, so waiting on it directly races the store.
            sp.wait_ge(red_sem, len(chunks))
            sp.wait_ge(act_sem, len(chunks) + len(pieces) + 1)
            sp.dma_start(z, ztile[:]).then_inc(out_sem, 16)
            sp.wait_ge(out_sem, 16)

        @block.scalar
        def _(act):
            # One batched in-place exp per DMA chunk (amortizes the 352-cycle
            # ACTIVATE pipeline cost); row-sums on DVE except the final tile,
            # where the ACT accumulator (0.28us) beats a DVE reduce (1.2us).
            # zero our own bias AP on ACT itself (replaces the const pool)
            act.memzero(bias0[:])
            done = 0
            for j, w in enumerate(chunks):
                act.wait_ge(in_sems[j], 16)
                sub = xbuf[:, done * C : (done + w) * C]
                act.activation(
                    sub, sub, mybir.ActivationFunctionType.Exp, bias=bias0[:]
                ).then_inc(act_sem, 1)
                done += w
            base = (NT - 1) * C
            for h, (o, wid) in enumerate(pieces):
                act.wait_ge(in_sems[len(chunks) + h], 16)
                sub = xbuf[:, base + o : base + o + wid]
                act.activation(
                    sub,
                    sub,
                    mybir.ActivationFunctionType.Exp,
                    bias=bias0[:],
                    accum_out=ztile[:, NT - 1 + h : NT + h],
                ).then_inc(act_sem, 1)
            # same-engine ordering: this tiny engine op completes only after
            # the READ_ACCUMULATOR stores above have retired
            act.mul(fence[:], fence[:], 0.0).then_inc(act_sem, 1)

        @block.vector
        def _(dve):
            # one batched reduce per exp'd chunk
            done = 0
            for k, w in enumerate(chunks):
                dve.wait_ge(act_sem, k + 1)
                dve.tensor_reduce(
                    ztile[:, done : done + w],
                    xbuf[:, done * C : (done + w) * C].rearrange(
                        "p (n c) -> p n c", c=C
                    ),
                    axis=mybir.AxisListType.X,
                    op=mybir.AluOpType.add,
                ).then_inc(red_sem, 1)
                done += w

    return nc


def _get_program():
    if "nc" not in _PROG_CACHE:
        _PROG_CACHE["nc"] = _build_program()
    return _PROG_CACHE["nc"]


def run_device(outputs_np, trace=False, trace_kwargs=None):
    """Run the Bass kernel on 8 cores; returns (Z[B] float32, BassKernelResults)."""
    from concourse.bass_utils import run_bass_kernel_spmd

    nc = _get_program()
    in_maps = [
        {"x": np.ascontiguousarray(outputs_np[r * B_LOCAL : (r + 1) * B_LOCAL])}
        for r in range(NCORES)
    ]
    kw = {}
    if trace:
        kw["trace"] = True
        if trace_kwargs:
            kw["trace_kwargs"] = trace_kwargs
    res = run_bass_kernel_spmd(nc, in_maps, list(range(NCORES)), **kw)
    zs = []
    for r in range(NCORES):
        zr = np.asarray(res.results[r]["z"])  # [P, NT+1]
        # last tile's sum arrives as two half-tile partial sums
        zfull = zr[:, :NT].copy()
        zfull[:, NT - 1] += zr[:, NT]
        zs.append(zfull)
    # z[p, i] corresponds to shard row i*P + p
    Z = np.concatenate([z.T.reshape(-1) for z in zs])  # [B]
    return Z, res


def _host_label_prep(outputs_np, labels_np):
    """Dedup weights, gathered logits, and per-row scale from the labels."""
    labels = labels_np.astype(np.int64)
    valid = labels != -1  # [B, K]
    num_comp = valid.sum(axis=1)  # [B]
    # first-occurrence mask: entry k is a dup if some j < k holds same value
    eq = labels[:, :, None] == labels[:, None, :]  # [B, K, K]
    earlier = np.arange(K)[None, :] < np.arange(K)[:, None]  # [K, K], (k, j): j<k
    is_dup = (eq & earlier[None, :, :]).any(axis=2)  # [B, K]
    w = valid & ~is_dup  # [B, K] bool
    safe = np.where(valid, labels, 0)
    g = outputs_np[np.arange(B)[:, None], safe]  # [B, K] f32 gathered logits
    return w, g, num_comp


def finish_loss(Z, w, g, num_comp):
    S = np.where(w, np.exp(g.astype(np.float64)), 0.0).sum(axis=1)  # [B]
    Z64 = Z.astype(np.float64)
    p_nc = (Z64 - S) / Z64
    loss = -np.log(p_nc + EPS)
    scale = (C - 1) / (C - num_comp.astype(np.float64))
    return np.asarray((scale * loss).mean(), dtype=np.float32)


def kernel(**inputs):
    outputs_np = np.ascontiguousarray(
        np.asarray(inputs["outputs"], dtype=np.float32)
    )
    labels_np = np.asarray(inputs["complementary_labels"])
    assert outputs_np.shape == (B, C)
    assert labels_np.shape == (B, K)

    w, g, num_comp = _host_label_prep(outputs_np, labels_np)
    Z, _ = run_device(outputs_np)
    return finish_loss(Z, w, g, num_comp)
